# revision 59
# baseline (speedup 1.0000x reference)
"""Chamfer loss (render points <-> full 256x256 pixel grid) on 8 TRN2 cores.

Math: for points p=(px,py) and pixel coords c=(x,y),
  d2[m,n] = ||c_m - p_n||^2 = x*(-2px) + y*(-2py) + cc*1 + 1*pp
computed as a single K=4 matmul per (128 pixel, 512 point) tile on the PE
(float32r = full-rate fp32). Since sqrt is monotonic, min(sqrt(d2)) =
sqrt(min(d2)): the DVE reduces d2 tiles to per-pixel minima and sqrt runs
only on the reduced values.

Term "sum over pixels of min over points" (min over dim=0): pixels are
sharded across the 8 cores (32 image rows each), fully local.
Term "sum over points of min over pixels" (min over dim=1): the minimum over
the FULL pixel lattice has a closed form (nearest lattice point, coordinates
clamped to [0,255], separable per axis); points are sharded 250/core.
Each core emits one scalar partial; the host sums the 8 partials.
"""

from contextlib import ExitStack

import numpy as np

import concourse.bacc as bacc
import concourse.bass as bass
import concourse.mybir as mybir
import concourse.tile as tile
from concourse import dve_ops
from concourse.bass_utils import run_bass_kernel_spmd
from concourse.dve_spec import AluOp, C0, Spec, Src0, Src1, lower, minn
from concourse.dve_uop import DveOpSpec


def _register_min2():
    """Custom DVE op: out = min(in0, in1); accum_out = min(out, s0 seed).
    Ingests two streams per cycle, so a (128, 2n) min-reduce costs n cycles
    instead of 2n. Registered at runtime into dve_ops.OPS."""
    name = "ANT_MIN2_REDUCE"
    for op in dve_ops.OPS:
        if op.name == name:
            return op
    def _ref(in0, in1, c0, c1, c2):
        b = np.minimum(in0.astype(np.float32), in1.astype(np.float32))
        return b, np.minimum(
            np.float32(c0), b.reshape(b.shape[0], -1).min(axis=-1, keepdims=True))

    spec = Spec(body=minn(Src0, Src1), accum=AluOp.MIN, accum_init=C0,
                reference=_ref)
    op = dve_ops.DveOp(name, spec, subdim=False, uops_sha={})
    for ver in ("v3", "v4"):
        s = DveOpSpec(name=name, opcode=0, uops=lower(spec, ver=ver),
                      rd1_en=True)
        op.uops_sha[ver] = s.sha(ver)
    row = max(dve_ops._SUB_OPCODE_FOR_NAME.values()) + 1
    assert row < 0x20
    dve_ops.OPS.append(op)
    dve_ops.CUSTOM_DVE_SPECS[name] = spec
    dve_ops._SUB_OPCODE_FOR_NAME[name] = row
    return op

H = 256
W = 256
N = 2000
NCORES = 8
ROWS_PER_CORE = H // NCORES          # 32
M_CORE = ROWS_PER_CORE * W           # 8192 pixels per core
NTILES = M_CORE // 128               # 64 pixel tiles per core
NPAD = 2048                          # point rows padded in DRAM layout
NREAL = 2000                         # points actually fed to the matmuls
PCHUNK = 512                         # moving-operand columns per matmul
NCHUNKS = (NREAL + PCHUNK - 1) // PCHUNK   # 4 (last chunk 464)
T1_PER_CORE = N // NCORES            # 250 (padded to 256 = 128x2)
FAR = 1.0e6                          # padding point coordinate (never the min)
KDIM = 9                             # 3 matmul terms x 3 bf16 splits each
                                     # (cc term added per-partition post-reduce)

_cache = {}


def _body(ctx, tc, nc, coords, pts, t1, cc_cols, out, reps=1):
    f32 = mybir.dt.float32
    bf16 = mybir.dt.bfloat16
    X = mybir.AxisListType.X
    alu = mybir.AluOpType

    singles = ctx.enter_context(tc.tile_pool(name="singles", bufs=1))
    psum_pool = ctx.enter_context(tc.tile_pool(name="psum", bufs=4, space="PSUM"))
    small = ctx.enter_context(tc.tile_pool(name="small", bufs=1))

    # ---- inputs -> SBUF ----
    # pts first (first matmul needs them), coords chunk 0 next; spread the
    # rest across different engines' DGE queues so they don't serialize.
    pts_sb = singles.tile([KDIM, NPAD], bf16)
    nc.sync.dma_start(pts_sb[:, 0:NREAL], pts[:, 0:NREAL])
    coords_sb = singles.tile([KDIM, M_CORE], bf16)
    CCH = M_CORE // 4
    qs = [nc.gpsimd, nc.sync, nc.gpsimd, nc.sync]
    for j in range(4):
        qs[j].dma_start(coords_sb[:, bass.ts(j, CCH)],
                        coords[:, bass.ts(j, CCH)])
    t1_sb = singles.tile([128, 4], f32)
    nc.gpsimd.dma_start(t1_sb[:], t1[:])
    cc_sb = singles.tile([128, NTILES], f32)
    nc.gpsimd.dma_start(cc_sb[:], cc_cols[:])

    # ---- main loop: d2 matmul tiles + per-pixel min reduce ----
    # Tiles alternate between two reduce paths to spread min work across
    # engines (DVE is otherwise the bottleneck):
    #   A: DVE tensor_reduce straight from PSUM
    #   B: ACT copies PSUM->SBUF, Pool folds min-tree 2000->125, DVE tail
    # dummy sqrt up front: pulls the Sqrt act-table load into the startup
    # bubble instead of the kernel tail
    warm = small.tile([1, 1], f32, tag="warm")
    nc.vector.memset(warm, 1.0)
    nc.scalar.activation(warm, warm, mybir.ActivationFunctionType.Sqrt)
    HALF = NREAL // 2
    # per-half minima land in pairbuf cols (2t, 2t+1); merged afterwards.
    # Plain PSUM->DVE reduces beat the ACT-copy+fused-min variant on HW by
    # ~50us: the copy chain serializes through the PSUM slots.
    pairbuf = singles.tile([128, 2 * NTILES], f32)
    minbuf = singles.tile([128, NTILES], f32)   # per-pixel d2 minima
    for _rep in range(reps):                    # reps>1 only for perf timing
        for t in range(NTILES):
            lhsT = coords_sb[:, bass.ts(t, 128)]    # (KDIM, 128) stationary
            psB = psum_pool.tile([128, HALF], f32, tag="ps")
            nc.tensor.matmul(psB[:, 0:PCHUNK], lhsT,
                             pts_sb[:, HALF:HALF + PCHUNK],
                             start=True, stop=True)
            nc.tensor.matmul(psB[:, PCHUNK:HALF], lhsT,
                             pts_sb[:, HALF + PCHUNK:NREAL],
                             start=True, stop=True)
            nc.vector.tensor_reduce(pairbuf[:, 2 * t + 1:2 * t + 2], psB[:],
                                    axis=X, op=alu.min)
            psA = psum_pool.tile([128, HALF], f32, tag="ps")
            nc.tensor.matmul(psA[:, 0:PCHUNK], lhsT, pts_sb[:, 0:PCHUNK],
                             start=True, stop=True)
            nc.tensor.matmul(psA[:, PCHUNK:HALF], lhsT,
                             pts_sb[:, PCHUNK:HALF],
                             start=True, stop=True)
            nc.vector.tensor_reduce(pairbuf[:, 2 * t:2 * t + 1], psA[:],
                                    axis=X, op=alu.min)
    pv = pairbuf[:].rearrange("p (t two) -> p t two", two=2)
    nc.vector.tensor_tensor(minbuf[:], pv[:, :, 0], pv[:, :, 1], op=alu.min)

    # ---- term1: exact distance to nearest lattice pixel, 256 pts/core ----
    # sq_in cols [0:NTILES) = relu(per-pixel minima); cols [NTILES:NTILES+2)
    # = per-point nearest-lattice d2 (exact, >= 0).
    sq_in = singles.tile([128, NTILES + 2], f32)
    d2pix = singles.tile([128, NTILES], f32)
    nc.vector.tensor_add(d2pix, minbuf[:], cc_sb[:])   # add back ||c||^2
    nc.vector.tensor_scalar_max(sq_in[:, 0:NTILES], d2pix[:], 0.0)

    # nearest lattice coordinate: t = RNE-round(v) via the 2^23 trick, then
    # the true clamped nearest is among {t-1, min(t,255), min(t+1,255)}.
    BIG = 8388608.0  # 2^23
    d2ax = []
    for a in range(2):                           # 0: x, 1: y
        v = t1_sb[:, 2 * a:2 * a + 2]            # (128, 2) coords
        t0 = small.tile([128, 2], f32, tag=f"t0{a}")
        nc.vector.tensor_scalar(t0, v, BIG, -BIG, op0=alu.add, op1=alu.add)
        cands = []
        cm = small.tile([128, 2], f32, tag=f"cm{a}")
        nc.vector.tensor_scalar(cm, t0, -1.0, None, op0=alu.add)
        cands.append(cm)
        c0 = small.tile([128, 2], f32, tag=f"c0{a}")
        nc.vector.tensor_scalar(c0, t0, 255.0, None, op0=alu.min)
        cands.append(c0)
        cp = small.tile([128, 2], f32, tag=f"cp{a}")
        nc.vector.tensor_scalar(cp, t0, 1.0, 255.0, op0=alu.add, op1=alu.min)
        cands.append(cp)
        sqs = []
        for i, c in enumerate(cands):
            df = small.tile([128, 2], f32, tag=f"df{a}{i}")
            nc.vector.tensor_sub(df, v, c)
            d2c = small.tile([128, 2], f32, tag=f"d2c{a}{i}")
            nc.vector.tensor_mul(d2c, df, df)
            sqs.append(d2c)
        m01 = small.tile([128, 2], f32, tag=f"m01{a}")
        nc.vector.tensor_tensor(m01, sqs[0], sqs[1], op=alu.min)
        d2 = small.tile([128, 2], f32, tag=f"d2{a}")
        nc.vector.tensor_tensor(d2, m01, sqs[2], op=alu.min)
        d2ax.append(d2)
    nc.vector.tensor_add(sq_in[:, NTILES:NTILES + 2], d2ax[0], d2ax[1])

    # ---- sqrt, row-sum, partition-sum (matmul with ones), store ----
    sq = singles.tile([128, NTILES + 2], f32)
    nc.scalar.activation(sq, sq_in, mybir.ActivationFunctionType.Sqrt)
    acc = singles.tile([128, 1], f32)
    nc.vector.tensor_reduce(acc, sq, axis=X, op=alu.add)
    ones = singles.tile([128, 1], f32)
    nc.vector.memset(ones, 1.0)
    ps_s = psum_pool.tile([1, 1], f32, tag="ps")
    nc.tensor.matmul(ps_s[:], acc[:], ones[:], start=True, stop=True)
    res = small.tile([1, 1], f32)
    nc.scalar.copy(res, ps_s)
    nc.sync.dma_start(out[0:1, 0:1], res)


MIN2 = _register_min2()


def _build_nc(reps=1):
    nc = bacc.Bacc(trn_type="TRN2", target_bir_lowering=False, debug=False)
    coords = nc.dram_tensor("coords_aug", [KDIM, M_CORE], mybir.dt.bfloat16,
                            kind="ExternalInput").ap()
    pts = nc.dram_tensor("pts_aug", [KDIM, NPAD], mybir.dt.bfloat16,
                         kind="ExternalInput").ap()
    t1 = nc.dram_tensor("t1xy", [128, 4], mybir.dt.float32,
                        kind="ExternalInput").ap()
    cc_cols = nc.dram_tensor("cc_cols", [128, NTILES], mybir.dt.float32,
                             kind="ExternalInput").ap()
    out = nc.dram_tensor("out", [1, 1], mybir.dt.float32,
                         kind="ExternalOutput").ap()
    with tile.TileContext(nc) as tc:
        with ExitStack() as ctx:
            _body(ctx, tc, nc, coords, pts, t1, cc_cols, out, reps=reps)
    nc.compile()
    return nc


def get_nc():
    if "nc" not in _cache:
        _cache["nc"] = _build_nc()
    return _cache["nc"]


def _split3(v):
    """Exact 3-way bf16 split of f32 values: v == s0 + s1 + s2 bitwise."""
    import ml_dtypes
    bf = ml_dtypes.bfloat16
    s0 = v.astype(bf)
    r1 = (v - s0.astype(np.float32)).astype(np.float32)
    s1 = r1.astype(bf)
    r2 = (r1 - s1.astype(np.float32)).astype(np.float32)
    s2 = r2.astype(bf)
    return s0, s1, s2


def make_in_maps(img_render_points, img_ref):
    import ml_dtypes
    bf = ml_dtypes.bfloat16
    pts = np.asarray(img_render_points, dtype=np.float32)
    px, py = pts[:, 0].copy(), pts[:, 1].copy()
    pp = px * px + py * py                      # matches reference's sum(p*p)

    # point-side rows (bf16): [-2px]x3, [-2py]x3, [pp]x3 (exact split sums)
    mx = np.full(NPAD, -2.0 * FAR, dtype=np.float32)
    my = np.full(NPAD, -2.0 * FAR, dtype=np.float32)
    mp = np.full(NPAD, 2.0 * FAR * FAR, dtype=np.float32)
    mx[:N] = -2.0 * px
    my[:N] = -2.0 * py
    mp[:N] = pp
    pts_aug = np.empty((KDIM, NPAD), dtype=bf)
    pts_aug[0:3] = np.stack(_split3(mx))
    pts_aug[3:6] = np.stack(_split3(my))
    pts_aug[6:9] = np.stack(_split3(mp))

    xs = np.tile(np.arange(W, dtype=np.float32), ROWS_PER_CORE)   # (8192,)
    in_maps = []
    for c in range(NCORES):
        ys = np.repeat(np.arange(c * ROWS_PER_CORE, (c + 1) * ROWS_PER_CORE,
                                 dtype=np.float32), W)
        cc = xs * xs + ys * ys                  # f32-exact (17-bit ints)
        coords_aug = np.empty((KDIM, M_CORE), dtype=bf)
        coords_aug[0] = xs.astype(bf)           # exact: integers <= 255
        coords_aug[1] = coords_aug[0]
        coords_aug[2] = coords_aug[0]
        coords_aug[3] = ys.astype(bf)
        coords_aug[4] = coords_aug[3]
        coords_aug[5] = coords_aug[3]
        coords_aug[6:9] = bf(1.0)
        cc_cols = cc.reshape(NTILES, 128).T.copy()   # (128, NTILES)

        sl = slice(c * T1_PER_CORE, (c + 1) * T1_PER_CORE)
        t1x = np.zeros(256, dtype=np.float32)
        t1y = np.zeros(256, dtype=np.float32)
        t1x[:T1_PER_CORE] = px[sl]
        t1y[:T1_PER_CORE] = py[sl]
        t1xy = np.empty((128, 4), dtype=np.float32)
        t1xy[:, 0:2] = t1x.reshape(2, 128).T    # col j holds pts j*128..j*128+127
        t1xy[:, 2:4] = t1y.reshape(2, 128).T

        in_maps.append({"coords_aug": coords_aug, "pts_aug": pts_aug,
                        "t1xy": t1xy, "cc_cols": cc_cols})
    return in_maps


def kernel(img_render_points, img_ref):
    nc = get_nc()
    in_maps = make_in_maps(img_render_points, img_ref)
    res = run_bass_kernel_spmd(nc, in_maps, core_ids=list(range(NCORES)))
    total = np.float32(np.sum(np.float64(
        [res.results[c]["out"][0, 0] for c in range(NCORES)])))
    return np.asarray(total, dtype=np.float32)
